# revision 48
# baseline (speedup 1.0000x reference)
"""AttentionConv2d Trainium2 kernel — 8-core batch-data-parallel (v3).

Each of the 8 NeuronCores processes one image of the batch:
  - qkv 1x1 conv (q,k only) + 3x3 conv as implicit-GEMM matmuls
  - V projection computed transposed (x as stationary) so the A*V stationary
    needs no on-device transposes
  - relative position logits G[y2,l]/W[x2,l] produced by 64 diagonal-packed
    matmuls (4 heads x 2 j-halves per pass via block-diagonal kr windows
    built on host)
  - per-head logits via contract-dim augmentation (32 k + 32 G-sel + 32 W-sel
    rows); logits/AV matmuls interleaved mb-wise to keep PE busy (p-state)
  - softmax exp on ScalarE from PSUM; denominators ride the A*V matmul as an
    appended ones-column; fast approx reciprocal + deferred division pipeline
  - final 1x1 conv; outputs concatenated [conv_out(256) ; attn(256)]
  - padded-image build + 3x3 conv run off the attention critical path
"""

import os
import sys

import numpy as np
import ml_dtypes

sys.path.insert(0, "/opt/trn_rl_repo")

B, C_IN, H, W = 8, 256, 32, 32
HW = H * W
DK = DV = 256
NH = 8
DKH = DK // NH  # 32
C_OUT = 512
N_CORES = 8

LOG2E = 1.4426950408889634

_CACHE = {}

# --- custom DVE exp2 (registered at import) --------------------------------

_DVE_MAGIC = 12582912.0  # 1.5 * 2^23
_DVE_BIAS = 127.0
_DVE_SCALE23 = 8388608.0  # 2^23
_DVE_POLY_A = 0.702941794  # linear coeff
_DVE_POLY_B = 0.239864029  # quadratic coeff


def _register_dve_exp2():
    import concourse.dve_ops as dve_ops_mod
    from concourse.dve_ops import DveOp
    from concourse.dve_spec import C0, C1, C2, One, Spec, Src0, Src1
    from concourse.dve_spec import _has_src1 as has_src1
    from concourse.dve_spec import lower
    from concourse.dve_table_gen import dve_ver_for
    from concourse.dve_uop import DveOpSpec

    if "EXP2_CARRIER_ANT" in dve_ops_mod._SUB_OPCODE_FOR_NAME:
        return tuple(
            next(o for o in dve_ops_mod.OPS if o.name == n)
            for n in ("EXP2_CARRIER_ANT", "EXP2_FINISH_ANT")
        )

    def f32(x):
        return np.asarray(x, np.float32)

    def ref_carrier(in0, in1, s0, s1, imm2):
        t = f32(in0)
        n = f32(f32(t + f32(s0)) - f32(s0))
        return f32(f32(n + f32(s1)) * f32(imm2))

    def ref_finish(in0, in1, s0, s1, imm2):
        t = f32(in0)
        n = f32(f32(t + f32(s0)) - f32(s0))
        f = f32(t - n)
        p = f32(f32(f32(f * f32(s1)) + f32(imm2)) * f + 1.0)
        return f32(p * f32(in1))

    u1 = Src0 + C0
    body_c = ((u1 - C0) + C1) * C2
    u2 = Src0 + C0
    f = Src0 - (u2 - C0)
    body_f = ((f * C1 + C2) * f + One) * Src1

    rows = sorted(dve_ops_mod._SUB_OPCODE_FOR_NAME.values())
    r1, r2 = rows[-1] + 1, rows[-1] + 2
    assert r2 < 0x20
    dve_ops_mod._SUB_OPCODE_FOR_NAME["EXP2_CARRIER_ANT"] = r1
    dve_ops_mod._SUB_OPCODE_FOR_NAME["EXP2_FINISH_ANT"] = r2

    ops = []
    for name, body, ref, row in (
        ("EXP2_CARRIER_ANT", body_c, ref_carrier, r1),
        ("EXP2_FINISH_ANT", body_f, ref_finish, r2),
    ):
        spec = Spec(body=body, reference=ref)
        ver = dve_ver_for("TRN2")
        tmp = DveOpSpec(
            name=name, opcode=row, uops=lower(spec, ver=ver),
            rd1_en=has_src1(spec),
        )
        op = DveOp(name, spec, subdim=False, uops_sha={ver: tmp.sha(ver)})
        dve_ops_mod.OPS.append(op)
        dve_ops_mod.CUSTOM_DVE_SPECS[name] = spec
        ops.append(op)
    return tuple(ops)



def _build():
    import concourse.bass as bass
    import concourse.mybir as mybir
    import concourse.tile as tile
    from concourse import bacc
    from contextlib import ExitStack

    op_carrier, op_finish = _register_dve_exp2()

    f32 = mybir.dt.float32
    bf16 = mybir.dt.bfloat16
    i32 = mybir.dt.int32
    AF = mybir.ActivationFunctionType

    nc = bacc.Bacc("TRN2", target_bir_lowering=False, debug=False,
                   num_devices=N_CORES)

    xc_d = nc.dram_tensor("xcb", [2, 128, HW], bf16, kind="ExternalInput").ap()
    wqkv_d = nc.dram_tensor("wqkvT", [2, 128, 768], bf16, kind="ExternalInput").ap()
    wout_d = nc.dram_tensor("woutT", [2, 128, 9, 256], bf16, kind="ExternalInput").ap()
    wattn_d = nc.dram_tensor("wattnT", [2, 128, 256], bf16, kind="ExternalInput").ap()
    khd_d = nc.dram_tensor("krhdiag", [128, 32, 128], bf16, kind="ExternalInput").ap()
    kwd_d = nc.dram_tensor("krwdiag", [128, 32, 128], bf16, kind="ExternalInput").ap()
    masks_d = nc.dram_tensor("masks", [64, 8, 128], bf16, kind="ExternalInput").ap()
    bqkv_d = nc.dram_tensor("bqkv", [128, 4], f32, kind="ExternalInput").ap()
    batt_d = nc.dram_tensor("battn", [128, 2], f32, kind="ExternalInput").ap()
    bout_d = nc.dram_tensor("bout", [128, 2], f32, kind="ExternalInput").ap()
    out_d = nc.dram_tensor("out", [4, 128, HW], f32, kind="ExternalOutput").ap()

    with tile.TileContext(nc) as tc, ExitStack() as ctx:
        wp = ctx.enter_context(tc.tile_pool(name="weights", bufs=1))
        ap_ = ctx.enter_context(tc.tile_pool(name="acts", bufs=1))
        hp = ctx.enter_context(tc.tile_pool(name="head", bufs=2))
        carp = ctx.enter_context(tc.tile_pool(name="carrier", bufs=2))
        pbig = ctx.enter_context(tc.tile_pool(name="pbig", bufs=3, space="PSUM"))

        # ---- weights / constants to SBUF ----
        wout = wp.tile([128, 2, 9, 256], bf16)
        wattn = wp.tile([128, 2, 256], bf16)
        khd = wp.tile([128, 32, 128], bf16)
        kwd = wp.tile([128, 32, 128], bf16)
        tmplt = [wp.tile([96, 8, 128], bf16, name=f"tmpl_{s}")
                 for s in range(2)]  # [contract, mb, jj] per k-slot
        bqkv = wp.tile([128, 4], f32)
        batt = wp.tile([128, 2], f32)
        bout = wp.tile([128, 2], f32)

        # ---- x uploaded bf16 (no on-device cast); per-j tiles so
        # dependency tracking doesn't serialize j=0 behind j=1 DMAs ----
        xct = [ap_.tile([128, HW], bf16, name=f"xc_{j}") for j in range(2)]
        wqt = [ap_.tile([128, 768], bf16, name=f"wqkv_{j}") for j in range(2)]
        xeng = [nc.sync, nc.scalar]
        for j in range(2):
            xeng[j].dma_start(wqt[j][:], wqkv_d[j])
        for j in range(2):
            for hh in range(2):
                xeng[j].dma_start(xct[j][:, hh * 512:(hh + 1) * 512],
                                  xc_d[j, :, hh * 512:(hh + 1) * 512])
        nc.sync.dma_start(bqkv[:], bqkv_d[:])
        for hh in range(2):
            nc.scalar.dma_start(khd[:, 16 * hh:16 * hh + 16, :],
                                khd_d[:, 16 * hh:16 * hh + 16, :])
            nc.scalar.dma_start(kwd[:, 16 * hh:16 * hh + 16, :],
                                kwd_d[:, 16 * hh:16 * hh + 16, :])
        for s in range(2):
            nc.gpsimd.dma_start(tmplt[s][32:96, :, :], masks_d[:])
        for j in range(2):
            nc.sync.dma_start(wattn[:, j, :], wattn_d[j])
            nc.sync.dma_start(wout[:, j, :, :], wout_d[j])
        nc.sync.dma_start(batt[:], batt_d[:])
        nc.sync.dma_start(bout[:], bout_d[:])

        # ---- qkv = Wqkv @ x (1x1 conv), q then k; vT interleaved ----
        qblk = ap_.tile([128, 2, 32, 32], bf16)
        kblk2 = ap_.tile([128, 2, 32, 32], bf16)
        vTe = ap_.tile([128, 8, 8, 33], bf16)  # [m, mb, h, d(+ones)]
        nc.gpsimd.memset(vTe[:], 1.0)

        def qkv_ob(ob):
            ps = pbig.tile([128, HW], f32, tag="big", name=f"qkvps{ob}")
            for half in range(2):
                for j in range(2):
                    nc.tensor.matmul(
                        ps[:, half * 512:(half + 1) * 512],
                        wqt[j][:, ob * 128:(ob + 1) * 128],
                        xct[j][:, half * 512:(half + 1) * 512],
                        start=(j == 0), stop=(j == 1),
                    )
            if ob < 2:
                dst = qblk[:, ob, :, :].rearrange("p y x -> p (y x)")
                nc.vector.tensor_scalar_add(dst, ps[:], bqkv[:, ob:ob + 1])
            else:
                dst = kblk2[:, ob - 2, :, :].rearrange("p y x -> p (y x)")
                nc.scalar.activation(dst, ps[:], AF.Identity,
                                     bias=bqkv[:, ob:ob + 1])

        def vT_half(hb):  # m-blocks 4*hb .. 4*hb+3
            pv = pbig.tile([128, HW], f32, tag="big")
            for bb in range(4):
                b = 4 * hb + bb
                for j in range(2):
                    nc.tensor.matmul(
                        pv[:, bb * 256:(bb + 1) * 256],
                        xct[j][:, 128 * b:128 * (b + 1)],
                        wqt[j][:, 512:768],
                        start=(j == 0), stop=(j == 1),
                    )
            nc.scalar.activation(
                vTe[:, 4 * hb:4 * hb + 4, :, 0:32],
                pv.rearrange("p (b h d) -> p b h d", b=4, h=8, d=32),
                AF.Copy)

        qkv_ob(0)
        qkv_ob(1)
        vT_half(0)
        vT_half(1)
        qkv_ob(2)
        qkv_ob(3)

        # ---- rel-position logits, 4 heads x 2 j per pass ----
        # G[32i+y2, j, y, x] = sum_d krh[31+y2-y, d] * q[32i+d, j, y, x]
        grelB = ap_.tile([128, 2, HW], bf16)
        wrelB = ap_.tile([128, 2, HW], bf16)
        if True:
            # G-side via pbig tiles in two y-halves (pmx pool dropped to
            # leave banks for pbig bufs=3)
            for yh in range(2):
                pgh = pbig.tile([128, HW], f32, tag="big")
                for yy in range(16):
                    y = 16 * yh + yy
                    nc.tensor.matmul(
                        pgh[:, yy * 64:(yy + 1) * 64], khd[:, y, :],
                        qblk[:, :, y, :],
                        start=True, stop=True,
                    )
                nc.scalar.activation(
                    grelB[:].rearrange("p j (y x) -> p j y x", y=32, x=32)
                    [:, :, 16 * yh:16 * yh + 16, :],
                    pgh.rearrange("p (y j x) -> p j y x", y=16, j=2, x=32),
                    AF.Copy)
            # W-side into pbig tiles so it does not wait on grelB staging
            for xh in range(2):
                pw = pbig.tile([128, HW], f32, tag="big")
                for xx in range(16):
                    x = 16 * xh + xx
                    nc.tensor.matmul(
                        pw[:, xx * 64:(xx + 1) * 64], kwd[:, x, :],
                        qblk[:, :, :, x],
                        start=True, stop=True,
                    )
                # pw cols are (x, j, y) -> (j, y, x)
                nc.vector.tensor_copy(
                    wrelB[:].rearrange("p j (y x) -> p j y x", y=32, x=32)
                    [:, :, :, 16 * xh:16 * xh + 16],
                    pw.rearrange("p (x j y) -> p j y x", x=16, j=2, y=32),
                )

        # ---- padded image + 3x3 conv blocks (interleaved into the
        # late head loop so their output DMAs land during the exp stream) ----
        xp = ap_.tile([128, 2, 34 * 34], bf16)
        nc.gpsimd.memset(xp[:], 0.0)
        for j in range(2):
            nc.gpsimd.tensor_copy(
                xp[:, j, :].rearrange("p (y x) -> p y x", y=34, x=34)[:, 1:33, 1:33],
                xct[j][:].rearrange("p (y x) -> p y x", y=32, x=32),
            )
        oconv = ap_.tile([128, 2, HW], f32)

        def xview(j, half, ky, kx):
            v = xp[:, j, :].rearrange("p (y x) -> p y x", y=34, x=34)
            return v[:, half * 16 + ky: half * 16 + ky + 16, kx: kx + 32]

        def conv3_ob(ob):
            ps = pbig.tile([128, HW], f32, tag="big", name=f"convps{ob}")
            for half in range(2):
                for j in range(2):
                    for t in range(9):
                        ky, kx = t // 3, t % 3
                        nc.tensor.matmul(
                            ps[:, half * 512:(half + 1) * 512],
                            wout[:, j, t, ob * 128:(ob + 1) * 128],
                            xview(j, half, ky, kx),
                            start=((j, t) == (0, 0)), stop=((j, t) == (1, 8)),
                        )
            if ob == 0:
                nc.vector.tensor_scalar_add(oconv[:, ob, :], ps[:],
                                            bout[:, ob:ob + 1])
            else:
                nc.scalar.activation(oconv[:, ob, :], ps[:], AF.Identity,
                                     bias=bout[:, ob:ob + 1])
            for hh in range(2):
                nc.sync.dma_start(out_d[ob, :, hh * 512:(hh + 1) * 512],
                                  oconv[:, ob, hh * 512:(hh + 1) * 512])

        # ---- per-head attention ----
        pav = ctx.enter_context(tc.tile_pool(name="pav", bufs=2, space="PSUM"))
        attn = ap_.tile([128, 2, HW], bf16)
        pend = []

        def divide(avp, hp0, j):
            dn = hp.tile([1, HW], f32, tag="dn")
            nc.vector.tensor_copy(dn[:, 0:512], avp[32:33, :])
            nc.vector.tensor_copy(dn[:, 512:1024], avp[96:97, :])
            rdn = hp.tile([1, HW], f32, tag="rdn")
            nc.vector.reciprocal_approx_fast(rdn[:], dn[:])
            rb = hp.tile([32, HW], f32, tag="rb")
            nc.gpsimd.partition_broadcast(rb[:], rdn[:])
            nc.vector.tensor_mul(attn[hp0:hp0 + 32, j, 0:512],
                                 avp[0:32, :], rb[:, 0:512])
            nc.vector.tensor_mul(attn[hp0:hp0 + 32, j, 512:1024],
                                 avp[64:96, :], rb[:, 512:1024])

        def av_mb(ctx_h, mb):
            st, avp, hh = ctx_h
            nc.tensor.matmul(
                avp[0:33, :], vTe[:, mb, hh, :], st[:, mb, 0:512],
                start=(mb == 0), stop=(mb == 7),
            )
            nc.tensor.matmul(
                avp[64:97, :], vTe[:, mb, hh, :], st[:, mb, 512:1024],
                start=(mb == 0), stop=(mb == 7),
            )

        prev = None  # (st, avp, h) with A6/A7 still to issue
        for h in range(NH):
            i, j, s = h % 4, h // 4, h % 2
            hp0 = 32 * i
            nc.vector.tensor_copy(
                tmplt[s][0:32, :, :],
                kblk2[hp0:hp0 + 32, j, :, :].rearrange("p y x -> p (y x)")
                .rearrange("p (m c) -> p m c", m=8, c=128),
            )
            rhs = hp.tile([96, HW], bf16, tag="rhs")
            nc.vector.tensor_copy(
                rhs[0:32, :],
                qblk[hp0:hp0 + 32, j, :, :].rearrange("p y x -> p (y x)"))
            nc.vector.tensor_copy(rhs[32:64, :], grelB[hp0:hp0 + 32, j, :])
            nc.vector.tensor_copy(rhs[64:96, :], wrelB[hp0:hp0 + 32, j, :])

            st = hp.tile([128, 8, HW], bf16, tag="st")
            avp = pav.tile([128, 512], f32, tag="av")
            cur = [st, avp, h]

            def logits_mb(mb):
                ps = pbig.tile([128, HW], f32, tag="big")
                for lh in range(2):
                    nc.tensor.matmul(
                        ps[:, lh * 512:(lh + 1) * 512],
                        tmplt[s][:, mb, :],
                        rhs[:, lh * 512:(lh + 1) * 512],
                        start=True, stop=True,
                    )
                if mb == 6:
                    car = carp.tile([128, HW], i32, tag="car")
                    nc.vector._custom_dve(
                        op_carrier, out=car[:], in0=ps[:],
                        s0=_DVE_MAGIC, s1=_DVE_BIAS, imm2=_DVE_SCALE23)
                    nc.vector._custom_dve(
                        op_finish, out=st[:, mb, :], in0=ps[:],
                        in1=car[:].bitcast(f32),
                        s0=_DVE_MAGIC, s1=_DVE_POLY_B, imm2=_DVE_POLY_A)
                else:
                    nc.scalar.activation(st[:, mb, :], ps[:], AF.Exp,
                                         scale=0.6931471805599453)

            # cross-head pipeline: finish the previous head's tail A*V inside
            # this head's slot stream, then its division, so PE never drains
            logits_mb(0)
            logits_mb(1)
            if prev is not None:
                av_mb(prev, 5)
                av_mb(prev, 6)
                av_mb(prev, 7)
                divide(*pend.pop())
            for mb in range(2, 8):
                if mb - 2 <= 4:
                    av_mb(cur, mb - 2)
                logits_mb(mb)
            pend.append((avp, hp0, j))
            prev = cur
            if h == 5:
                conv3_ob(0)
            elif h == 6:
                conv3_ob(1)

        av_mb(prev, 5)
        av_mb(prev, 6)
        av_mb(prev, 7)
        # attn-conv j=0 halves need only heads 0-3 (divided long ago):
        # pre-issue them so only the j=1 halves trail the final division
        psa = [pbig.tile([128, HW], f32, tag="big", name=f"attnps{ob}")
               for ob in range(2)]
        for ob in range(2):
            for lh in range(2):
                nc.tensor.matmul(
                    psa[ob][:, lh * 512:(lh + 1) * 512],
                    wattn[:, 0, ob * 128:(ob + 1) * 128],
                    attn[:, 0, lh * 512:(lh + 1) * 512],
                    start=True, stop=False,
                )
        if pend:
            divide(*pend.pop())

        # ---- epilogue: attn 1x1 conv only (3x3 conv already issued) ----
        oattn = ap_.tile([128, 2, HW], f32)


        def attnconv_ob(ob):
            ps = psa[ob]
            for lh in range(2):
                nc.tensor.matmul(
                    ps[:, lh * 512:(lh + 1) * 512],
                    wattn[:, 1, ob * 128:(ob + 1) * 128],
                    attn[:, 1, lh * 512:(lh + 1) * 512],
                    start=False, stop=True,
                )
            nc.vector.tensor_scalar_add(oattn[:, ob, :], ps[:],
                                        batt[:, ob:ob + 1])
            for hh in range(2):
                nc.scalar.dma_start(out_d[2 + ob, :, hh * 512:(hh + 1) * 512],
                                    oattn[:, ob, hh * 512:(hh + 1) * 512])

        attnconv_ob(0)
        attnconv_ob(1)

    nc.compile()
    return nc


def _host_inputs(x, w_qkv, b_qkv, w_attn, b_attn, w_out, b_out,
                 key_rel_w, key_rel_h):
    bf = ml_dtypes.bfloat16
    s = DKH ** -0.5
    wq = np.asarray(w_qkv, np.float32)[:, :, 0, 0].copy()   # [768, 256]
    bq = np.asarray(b_qkv, np.float32).copy()
    wq[:DK] *= s
    bq[:DK] *= s
    wq[DK:2 * DK] *= LOG2E
    bq[DK:2 * DK] *= LOG2E
    wqkvT = np.ascontiguousarray(wq.T).reshape(2, 128, 768).astype(bf)
    wa = np.asarray(w_attn, np.float32)[:, :, 0, 0]          # [256, 256]
    wattnT = np.ascontiguousarray(wa.T).reshape(2, 128, 256).astype(bf)
    woutT = np.ascontiguousarray(
        np.asarray(w_out, np.float32).transpose(1, 2, 3, 0).reshape(256, 9, 256)
    ).reshape(2, 128, 9, 256).astype(bf)

    # block-diagonal shifted windows: diag[32i+d, y, 32i+y2] = krX[31+y2-y, d]
    def diag_windows(kr):
        krT = np.ascontiguousarray(np.asarray(kr, np.float32).T) * LOG2E
        idx = 31 + np.arange(32)[None, :] - np.arange(32)[:, None]  # [y, y2]
        base = krT[:, idx]                                   # [32d, 32y, 32y2]
        A = np.zeros((4, 32, 32, 4, 32), np.float32)
        for i in range(4):
            A[i, :, :, i, :] = base
        return np.ascontiguousarray(A.reshape(128, 32, 128)).astype(bf)

    krhdiag = diag_windows(key_rel_h)
    krwdiag = diag_windows(key_rel_w)

    masks = np.zeros((64, 8, 128), np.float32)
    for mb in range(8):
        for jj in range(128):
            masks[(mb * 4 + jj // 32) % 32, mb, jj] = 1.0  # U32 (y2 rows 0:32)
    for jj in range(128):
        masks[32 + jj % 32, :, jj] = 1.0                   # I32 (x2 rows 32:64)
    masks = masks.astype(bf)

    bqkv = np.ascontiguousarray(bq[:512].reshape(4, 128).T)           # [128, 4]
    bv = np.asarray(b_qkv, np.float32)[512:768]
    battn = np.asarray(b_attn, np.float32) + wa @ bv       # fold v-bias
    battn = np.ascontiguousarray(battn.reshape(2, 128).T)
    boutm = np.ascontiguousarray(np.asarray(b_out, np.float32).reshape(2, 128).T)

    shared = dict(wqkvT=wqkvT, wattnT=wattnT, woutT=woutT, krhdiag=krhdiag,
                  krwdiag=krwdiag, masks=masks, bqkv=bqkv, battn=battn,
                  bout=boutm)
    xs = np.asarray(x, np.float32).reshape(B, 2, 128, HW).astype(bf)
    return [dict(shared, xcb=np.ascontiguousarray(xs[i]))
            for i in range(N_CORES)]


def kernel(**inputs):
    from concourse.bass_utils import run_bass_kernel_spmd
    if "nc" not in _CACHE:
        _CACHE["nc"] = _build()
    nc = _CACHE["nc"]
    in_maps = _host_inputs(**inputs)
    res = run_bass_kernel_spmd(nc, in_maps, list(range(N_CORES)),
                               trace=bool(os.environ.get("BASS_KERNEL_TRACE")))
    _CACHE["last_result"] = res
    outs = [r["out"].reshape(C_OUT, H, W) for r in res.results]
    return np.stack(outs).astype(np.float32)

